# revision 29
# baseline (speedup 1.0000x reference)
"""Trainium2 Bass kernel for BoundaryPredictor2 (B=4, L=1024, D=512, H=8).

Sharding: 8 cores = 4 batch rows x 2 token-halves (512 tokens each).

v3: ONE merged pairwise AllGather carries z' (2080 f16 cols) + hard
boundary bits + straddle columns (12 f16 cols), all partition-major so
every staging/download DMA is row-contiguous (no 4-byte scatter packets
-- those were throttling the collective's own 33KB packets 7x).

  Phase order: l2norm(hidden) [FR + f16 twin] -> W1 fp16 -> LN stats
  (fp32r reductions) -> z' fp16 (hn@WpvT * exp-scores) -> W2 fp16 +
  fp32r residual -> l2norm -> fold g = (Wq^T Wk)^T u, one fp32r pass
  (margins verified 4x in fp64 sim) -> cos via fp32r ones-reduction ->
  hard -> PE-transpose to [128,4] -> merged AllGather -> seg ids in
  [128,8] chunk layout -> one-hot pooling (per-feature-chunk PSUM
  banks) -> normalize -> Wpo matmul (doubles as [feat,slot]->[slot,feat]
  transpose) -> out.

  Consts are DMA'd before the 1MB hidden slice so the FR twins and the
  first l2norm matmul aren't gated on the weight queue; each D x D
  weight is ONE rearranged DMA (sync-engine enqueues cost ~600ns each).
"""

from contextlib import ExitStack

import numpy as np

import concourse.bass as bass
import concourse.tile as tile
from concourse import bacc, mybir
from concourse.bass_utils import run_bass_kernel_spmd

FP = mybir.dt.float32
FR = mybir.dt.float32r
F16 = mybir.dt.float16
AF = mybir.ActivationFunctionType
OP = mybir.AluOpType
AX = mybir.AxisListType

N_CORES = 8
B, L, D = 4, 1024, 512
H, HD = 8, 64
TPC = 512           # own tokens per core
NCH = D // 128      # feature chunks (4)
SCALE = HD ** -0.5
ZF = D + H          # z feature width (512 V cols + 8 e cols)
CW = 4 * ZF + 12    # merged collective width: z' + hardT(4) + u0(4) + g511(4)
TG = ((0, 256), (256, 256))   # token pipeline groups

_CACHE = {}


def _emit(nc, tc, prm, out):
    ctx = ExitStack()
    cpool = ctx.enter_context(tc.tile_pool(name="consts", bufs=1))
    wpool = ctx.enter_context(tc.tile_pool(name="weights", bufs=1))
    apool = ctx.enter_context(tc.tile_pool(name="acts", bufs=1))
    spool = ctx.enter_context(tc.tile_pool(name="scratch", bufs=2))
    rpool = ctx.enter_context(tc.tile_pool(name="rows", bufs=1))
    psm = ctx.enter_context(tc.tile_pool(name="psm", bufs=2, space="PSUM"))
    pbig = ctx.enter_context(tc.tile_pool(name="pbig", bufs=4, space="PSUM"))
    dpool = ctx.enter_context(tc.tile_pool(name="dram", bufs=1, space="DRAM"))

    def ps_small():
        return psm.tile([128, 512], FP, tag="ps1", name="ps1")

    def ps_big():
        return pbig.tile([128, 512], FP, tag="pb", name="pb")

    def dma(dst, src):
        nc.sync.dma_start(out=dst, in_=src)

    def load(pool, name, shape, dt=FP, tag=None):
        t = pool.tile(list(shape), dt, tag=tag or name, name=name)
        dma(t[:], prm[name])
        return t

    pairs = [[2 * i, 2 * i + 1] for i in range(N_CORES // 2)]

    # ---- warmup collective: doorbell ASAP (no input-queue deps) to
    # absorb the CC stream's ~11us cold first-op pickup latency ----
    wz = cpool.tile([1, 1], FP, tag="wz", name="wz")
    nc.gpsimd.memset(wz[:], 0.0)
    wui = dpool.tile([1, 1], FP, tag="wui", name="wui")
    wuo = dpool.tile([2, 1], FP, tag="wuo", name="wuo")
    nc.sync.dma_start(out=wui[:], in_=wz[:])
    nc.gpsimd.collective_compute(
        "AllGather", OP.bypass, replica_groups=pairs,
        ins=[wui.opt()], outs=[wuo.opt()])

    # DMA enqueue instructions cost ~600ns EACH and are serial per
    # engine; spread them: sync = critical stream (hidden, big weights,
    # collective, outputs), scalar = early small consts, gpsimd =
    # pool-phase consts.
    def sload(name, shape, dt=FP):
        t = cpool.tile(list(shape), dt, tag=name, name=name)
        nc.scalar.dma_start(out=t[:], in_=prm[name])
        return t

    def gload(name, shape, dt=FP, src=None):
        t = cpool.tile(list(shape), dt, tag=name, name=name)
        nc.gpsimd.dma_start(out=t[:], in_=src if src is not None
                            else prm[name])
        return t

    # ---- hidden slice on the sync queue immediately ----
    xT = [apool.tile([128, TPC], FR, tag=f"xT{c}", name=f"xT{c}")
          for c in range(NCH)]
    for c in range(NCH):
        dma(xT[c][:], prm["hT"][c * 128:(c + 1) * 128, :])

    ones128 = sload("ones128", (128, 1))
    ones_r = sload("ones_r", (1, 128))
    ident = sload("ident", (128, 128))
    bias1 = sload("bias1", (128, NCH))
    lng = sload("lng", (128, NCH))
    lnb = sload("lnb", (128, NCH))
    bias2 = sload("bias2", (128, NCH))
    lenmask = sload("lenmask", (128, 4))
    simb = sload("simb", (1, 1))
    vm8 = gload("vm8", (128, 8))
    st8 = gload("st8", (128, 8))
    strc = gload("strc", (1, 1))
    ltri = gload("ltri", (128, 128))
    iota512 = gload("iota512", (128, TPC), dt=F16)
    e8 = gload("e8", (8, D), dt=F16)
    wkeff = cpool.tile([128, NCH * H], F16, tag="wkeff", name="wkeff")
    for c in range(NCH):
        nc.gpsimd.dma_start(out=wkeff[:, c * H:(c + 1) * H],
                            in_=prm["wkeff"][c * 128:(c + 1) * 128, :])

    # ---- weights per chunk: sync queue (after xT), wpot on gpsimd ----
    wt = {}
    for w, wdt in (("w1t", F16), ("wpvt", F16), ("w2t", F16),
                   ("wgt", FR), ("wpot", F16)):
        wt[w] = [wpool.tile([128, D], wdt, tag=f"{w}{c}", name=f"{w}{c}")
                 for c in range(NCH)]
        for c in range(NCH):
            src = prm[w][c * 128:(c + 1) * 128, :]
            if w == "wpot":
                nc.gpsimd.dma_start(out=wt[w][c][:], in_=src)
            else:
                dma(wt[w][c][:], src)

    # FR twins (fp32r matmul operands must be produced as fp32r)
    ones128r = cpool.tile([128, 1], FR, tag="ones128r", name="ones128r")
    nc.vector.tensor_copy(ones128r[:], ones128[:])
    ones_rr = cpool.tile([1, 128], FR, tag="ones_rr", name="ones_rr")
    nc.vector.tensor_copy(ones_rr[:], ones_r[:])
    identr = cpool.tile([128, 128], FR, tag="identr", name="identr")
    nc.vector.tensor_copy(identr[:], ident[:])

    def col(t, c):
        return t[:, c:c + 1]

    def rsqrt_row(dst, src, eps, mode, fr=False, w=TPC):
        sm = rpool.tile([1, TPC], FP, tag="rs_sm", name="rs_sm",
                        bufs=3)[0:1, 0:w]
        nc.vector.tensor_scalar(out=sm, in0=src, scalar1=eps,
                                scalar2=None,
                                op0=(OP.max if mode == "clip" else OP.add))
        sqv = rpool.tile([1, TPC], FP, tag="rs_sq", name="rs_sq",
                         bufs=3)[0:1, 0:w]
        nc.scalar.activation(sqv, sm, AF.Sqrt)
        if fr:
            r0 = rpool.tile([1, TPC], FP, tag="rs_r0", name="rs_r0",
                            bufs=3)[0:1, 0:w]
            nc.vector.reciprocal_approx_fast(r0, sqv)
            nc.vector.tensor_copy(dst, r0)
        else:
            nc.vector.reciprocal_approx_fast(dst, sqv)

    def l2norm_fm(src_tiles, dst_tiles, msq_keep=None, twin16=None):
        # ALL stats first (g1's squares don't sit behind g0's normalize
        # in the vector queue), then the per-group normalize blocks.
        pss = {}
        for g, (g0, gn) in enumerate(TG):
            ps = ps_small()
            for c in range(NCH):
                sq = spool.tile([128, 256], FR, tag="sq", name="sq", bufs=4)
                nc.vector.tensor_mul(sq[:], src_tiles[c][:, g0:g0 + gn],
                                     src_tiles[c][:, g0:g0 + gn])
                nc.tensor.matmul(ps[0:1, 0:gn], ones128r[:], sq[:],
                                 start=(c == 0), stop=(c == NCH - 1))
            pss[g] = ps
        for g, (g0, gn) in enumerate(TG):
            r = rpool.tile([1, 256], FR, tag="nrm_r", name="nrm_r", bufs=2)
            rsqrt_row(r[0:1, :], pss[g][0:1, 0:gn], 1e-16, "clip", fr=True,
                      w=gn)
            rb = ps_big()
            nc.tensor.matmul(rb[:, 0:gn], ones_rr[:], r[0:1, :],
                             start=True, stop=True)
            for c in range(NCH):
                nc.vector.tensor_mul(dst_tiles[c][:, g0:g0 + gn],
                                     src_tiles[c][:, g0:g0 + gn],
                                     rb[:, 0:gn])
                if twin16 is not None:
                    nc.scalar.copy(twin16[c][:, g0:g0 + gn],
                                   dst_tiles[c][:, g0:g0 + gn])
        if msq_keep is not None:
            nc.vector.tensor_copy(msq_keep[0:1, 0:256], pss[0][0:1, 0:256])
            nc.vector.tensor_copy(msq_keep[0:1, 256:512], pss[1][0:1, 0:256])

    # ---- l2norm of hidden: FR copy (residual) + f16 twin (W1 rhs) ----
    hn2r = [apool.tile([128, TPC], FR, tag=f"hn2r_{c}", name=f"hn2r_{c}")
            for c in range(NCH)]
    hn16 = [apool.tile([128, TPC], F16, tag=f"hn16_{c}", name=f"hn16_{c}")
            for c in range(NCH)]
    msq = rpool.tile([1, TPC], FP, tag="msq", name="msq")[0:1, :]
    l2norm_fm(xT, hn2r, msq_keep=msq, twin16=hn16)

    t1 = [apool.tile([128, TPC], F16, tag=f"t1_{c}", name=f"t1_{c}")
          for c in range(NCH)]

    def w1_pass(g):
        g0, gn = TG[g]
        for ech in range(NCH):
            ps = ps_big()
            for c in range(NCH):
                nc.tensor.matmul(ps[:, 0:gn],
                                 wt["w1t"][c][:, ech * 128:(ech + 1) * 128],
                                 hn16[c][:, g0:g0 + gn],
                                 start=(c == 0), stop=(c == NCH - 1))
            nc.scalar.activation(t1[ech][:, g0:g0 + gn], ps[:, 0:gn],
                                 AF.Gelu, bias=col(bias1, ech))

    w1_pass(0)

    # ---- layernorm stats (fp32r reductions) ----
    mups = ps_small()
    for c in range(NCH):
        nc.tensor.matmul(mups[0:1, :], ones128r[:], xT[c][:],
                         start=(c == 0), stop=(c == NCH - 1))
    mu = rpool.tile([1, TPC], FR, tag="mu", name="mu")[0:1, :]
    nc.vector.tensor_scalar(out=mu, in0=mups[0:1, :], scalar1=1.0 / D,
                            scalar2=None, op0=OP.mult)
    var = rpool.tile([1, TPC], FP, tag="var", name="var")[0:1, :]
    nc.vector.tensor_scalar(out=var, in0=msq, scalar1=1.0 / D,
                            scalar2=None, op0=OP.mult)
    mu2 = rpool.tile([1, TPC], FP, tag="mu2", name="mu2")[0:1, :]
    nc.vector.tensor_mul(mu2, mu, mu)
    nc.vector.tensor_sub(var, var, mu2)
    rstd = rpool.tile([1, TPC], FR, tag="rstd", name="rstd")[0:1, :]
    rsqrt_row(rstd, var, 1e-5, "add", fr=True)
    mub = ps_big()
    nc.tensor.matmul(mub[:], ones_rr[:], mu, start=True, stop=True)
    rstdb = ps_big()
    nc.tensor.matmul(rstdb[:], ones_rr[:], rstd, start=True, stop=True)
    w1_pass(1)
    hn = [apool.tile([128, TPC], F16, tag=f"hn_{c}", name=f"hn_{c}")
          for c in range(NCH)]
    for c in range(NCH):
        ht = spool.tile([128, TPC], FP, tag="htmp", name="htmp")
        nc.vector.tensor_sub(ht[:], xT[c][:], mub[:])
        nc.vector.tensor_mul(ht[:], ht[:], rstdb[:])
        nc.scalar.activation(hn[c][:], ht[:], AF.Identity,
                             bias=col(lnb, c), scale=col(lng, c))

    # ---- W2 + fp32r residual -> v; l2norm -> u (FR) ----
    v = [apool.tile([128, TPC], FP, tag=f"v_{c}", name=f"v_{c}")
         for c in range(NCH)]
    for g, (g0, gn) in enumerate(TG):
        for ech in range(NCH):
            ps = ps_big()
            for c in range(NCH):
                nc.tensor.matmul(ps[:, 0:gn],
                                 wt["w2t"][c][:, ech * 128:(ech + 1) * 128],
                                 t1[c][:, g0:g0 + gn],
                                 start=(c == 0), stop=False)
            nc.tensor.matmul(ps[:, 0:gn], identr[:],
                             hn2r[ech][:, g0:g0 + gn],
                             start=False, stop=True)
            nc.scalar.add(v[ech][:, g0:g0 + gn], ps[:, 0:gn],
                          col(bias2, ech))

    u = [apool.tile([128, TPC], FR, tag=f"u_{c}", name=f"u_{c}")
         for c in range(NCH)]
    l2norm_fm(v, u)

    # ---- z' = (hn@WpvT) * e per token chunk, fp16; one SBUF tile ----
    zall = apool.tile([128, 4 * ZF], F16, tag="zall", name="zall")
    for tch in range(4):
        tsl = slice(tch * 128, (tch + 1) * 128)
        z0 = tch * ZF
        scps = ps_small()
        for c in range(NCH):
            nc.tensor.matmul(scps[:, 0:H], hn[c][:, tsl],
                             wkeff[:, c * H:(c + 1) * H],
                             start=(c == 0), stop=(c == NCH - 1))
        e = spool.tile([128, H], FP, tag="e", name="e")
        nc.scalar.activation(e[:], scps[:, 0:H], AF.Exp, scale=SCALE)
        nc.gpsimd.tensor_scalar(out=e[:], in0=e[:],
                                scalar1=lenmask[:, tch:tch + 1], scalar2=None,
                                op0=OP.mult)
        vp = ps_big()
        for c in range(NCH):
            nc.tensor.matmul(vp[:], hn[c][:, tsl], wt["wpvt"][c][:],
                             start=(c == 0), stop=(c == NCH - 1))
        nc.vector.tensor_tensor(
            out=zall[:, z0:z0 + D].rearrange("p (h d) -> p h d", h=H),
            in0=vp[:].rearrange("p (h d) -> p h d", h=H),
            in1=e[:].broadcast_to([128, H, HD]),
            op=OP.mult)
        nc.gpsimd.tensor_copy(zall[:, z0 + D:z0 + ZF], e[:])

    # merged-collective input: z' cols now, boundary cols after the chain
    czi = dpool.tile([128, CW], F16, tag="czi", name="czi")
    czo = dpool.tile([256, CW], F16, tag="czo", name="czo")
    dma(czi[:, 0:4 * ZF], zall[:])

    # ---- fold pass: g = Wg^T u (fp32r, free=512); cos accumulation ----
    # hs16 staging tile: cols 0:4 hardT, 4:8 u col0, 8:12 g col511.
    hs16 = rpool.tile([128, 12], F16, tag="hs16", name="hs16")
    for ech in range(NCH):
        nc.vector.tensor_copy(hs16[:, 4 + ech:5 + ech], u[ech][:, 0:1])
    cosps = ps_small()
    for ech in range(NCH):
        psg = ps_big()
        for c in range(NCH):
            nc.tensor.matmul(psg[:],
                             wt["wgt"][c][:, ech * 128:(ech + 1) * 128],
                             u[c][:],
                             start=(c == 0), stop=(c == NCH - 1))
        nc.vector.tensor_copy(hs16[:, 8 + ech:9 + ech],
                              psg[:, TPC - 1:TPC])
        pr = spool.tile([128, TPC], FR, tag="prod", name="prod")
        nc.vector.tensor_mul(pr[:, 0:TPC - 1], psg[:, 0:TPC - 1],
                             u[ech][:, 1:TPC])
        nc.vector.tensor_scalar(out=pr[:, TPC - 1:TPC],
                                in0=psg[:, TPC - 1:TPC], scalar1=0.0,
                                scalar2=None, op0=OP.mult)
        nc.tensor.matmul(cosps[0:1, :], ones128r[:], pr[:],
                         start=(ech == 0), stop=(ech == NCH - 1))
    sgn = rpool.tile([1, TPC], FP, tag="sgn", name="sgn")[0:1, :]
    nc.scalar.activation(sgn, cosps[0:1, :], AF.Sign, bias=simb[0:1, 0:1])
    hard = rpool.tile([1, TPC], FP, tag="hard", name="hard")[0:1, :]
    nc.scalar.activation(hard, sgn, AF.Relu, scale=-1.0)
    # PE-transpose hard [1,512] -> [128,4] so the staging DMA is
    # row-contiguous (4-byte scatter DMAs throttle the collective 7x).
    psH = ps_small()
    for c in range(NCH):
        nc.tensor.transpose(psH[:, c:c + 1],
                            hard[0:1, c * 128:(c + 1) * 128],
                            ident[0:1, 0:1])
    nc.vector.tensor_copy(hs16[:, 0:4], psH[:, 0:4])

    dma(czi[:, 4 * ZF:CW], hs16[:])
    nc.gpsimd.collective_compute(
        "AllGather", OP.bypass, replica_groups=pairs,
        ins=[czi.opt()], outs=[czo.opt()])

    # ---- downloads: all row-contiguous, rank-indexed == global order ----
    zpa = apool.tile([128, 4 * ZF], F16, tag="zpa", name="zpa")
    hbA = rpool.tile([128, 12], F16, tag="hbA", name="hbA")  # rank0 cols
    hbB = rpool.tile([128, 12], F16, tag="hbB", name="hbB")  # rank1 cols
    dma(hbA[:], czo[0:128, 4 * ZF:CW])
    dma(hbB[:], czo[128:256, 4 * ZF:CW])
    dma(zpa[:], czo[128:256, 0:4 * ZF])

    def zf_v(k, c):
        t = zall if k < 4 else zpa
        base = (k % 4) * ZF
        return t[:, base + c * 128:base + (c + 1) * 128]

    def zf_e(k):
        t = zall if k < 4 else zpa
        base = (k % 4) * ZF
        return t[:, base + D:base + ZF]

    # ---- seg ids in [128, 8] token-chunk layout ----
    w12 = rpool.tile([128, 12], FP, tag="w12", name="w12")
    nc.vector.tensor_mul(w12[:, 0:4], hbA[:, 0:4], vm8[:, 0:4])
    nc.vector.tensor_mul(w12[:, 4:8], hbB[:, 0:4], vm8[:, 4:8])
    nc.vector.tensor_max(w12[:, 0:8], w12[:, 0:8], st8[:])
    # straddle products: g511_rank0 * u0_rank1 (feature dot, 4 col chunks)
    nc.vector.tensor_mul(w12[:, 8:12], hbA[:, 8:12], hbB[:, 4:8])
    totp = ps_small()
    nc.tensor.matmul(totp[0:1, 0:12], ones128[:], w12[:],
                     start=True, stop=True)
    scos = rpool.tile([1, 1], FP, tag="scos", name="scos")
    nc.vector.reduce_sum(scos[:], totp[0:1, 8:12], axis=AX.X)
    nc.scalar.activation(scos[:], scos[:], AF.Sign, bias=simb[0:1, 0:1])
    sdel = rpool.tile([1, 1], FP, tag="sdel", name="sdel")
    nc.scalar.activation(sdel[:], scos[:], AF.Relu, scale=-1.0)
    nc.vector.tensor_scalar(out=sdel[:], in0=sdel[:],
                            scalar1=strc[0:1, 0:1], scalar2=None,
                            op0=OP.mult)

    tot = rpool.tile([1, 8], FP, tag="tot", name="tot")
    nc.vector.tensor_copy(tot[:], totp[0:1, 0:8])
    offs = rpool.tile([1, 8], FP, tag="offs", name="offs")
    nc.vector.tensor_tensor_scan(offs[:], tot[:], tot[:], 0.0,
                                 OP.add, OP.bypass)
    nc.vector.tensor_sub(offs[:], offs[:], tot[:])      # exclusive prefix
    nc.vector.tensor_scalar(out=offs[0:1, 4:8], in0=offs[0:1, 4:8],
                            scalar1=sdel[0:1, 0:1], scalar2=None,
                            op0=OP.add)
    segp = ps_small()
    nc.tensor.matmul(segp[:, 0:8], ltri[:], w12[:, 0:8],
                     start=True, stop=False)
    nc.tensor.matmul(segp[:, 0:8], ones_r[:], offs[:],
                     start=False, stop=True)
    sego = rpool.tile([128, 8], FP, tag="sego", name="sego")
    nc.vector.tensor_copy(sego[:], segp[:, 0:8])

    # ---- pooling: one-hots, denominators, psA per feature chunk ----
    Af = [apool.tile([128, TPC], F16,
                     tag=f"xT{k}" if k < 4 else f"t1_{k - 4}",
                     name=f"Af_{k}") for k in range(8)]
    for k in range(8):
        nc.vector.tensor_scalar(out=Af[k][:], in0=iota512[:],
                                scalar1=sego[:, k:k + 1], scalar2=None,
                                op0=OP.is_equal)

    # psE (denominators) first, then psA keeps the PE busy while the
    # vector/scalar engines turn psE into per-chunk reciprocals.
    psE = psm.tile([128, 512], FP, tag="psE", name="psE", bufs=1)
    for k in range(8):
        nc.tensor.matmul(psE[0:H, :], zf_e(k), Af[k][:],
                         start=(k == 0), stop=(k == 7))
    psA = [pbig.tile([128, 512], FP, tag="pb", name=f"psA_{c}")
           for c in range(NCH)]
    for c in range(NCH):
        for k in range(8):
            nc.tensor.matmul(psA[c][:], zf_v(k, c), Af[k][:],
                             start=(k == 0), stop=(k == 7))
    dd = spool.tile([8, TPC], FP, tag="dd", name="dd")
    nc.vector.tensor_scalar(out=dd[:], in0=psE[0:H, :], scalar1=0.0,
                            scalar2=None, op0=OP.is_equal)
    nc.vector.tensor_add(dd[:], dd[:], psE[0:H, :])
    rec = spool.tile([8, TPC], FP, tag="rec", name="rec")
    nc.vector.reciprocal_approx_fast(rec[:], dd[:])
    rec16 = spool.tile([8, TPC], F16, tag="rec16", name="rec16")
    nc.vector.tensor_copy(rec16[:], rec[:])
    recs = [apool.tile([128, TPC], F16, tag=f"v_{c}", name=f"recs_{c}")
            for c in range(NCH)]
    for c in range(NCH):
        psR = psm.tile([128, 512], FP, tag="psR", name="psR", bufs=1)
        nc.tensor.matmul(psR[:], e8[:, c * 128:(c + 1) * 128], rec16[:],
                         start=True, stop=True)
        nc.scalar.copy(recs[c][:], psR[:])

    pn = [apool.tile([128, TPC], F16, tag=f"hn2r_{c}", name=f"pn_{c}")
          for c in range(NCH)]
    for c in range(NCH):
        nc.vector.tensor_mul(pn[c][:], psA[c][:], recs[c][:])

    # psO[slot, feat_out] = sum_c pn_c[:, jsl].T @ WpoT_c (transposes too)
    for j in range(4):
        jsl = slice(j * 128, (j + 1) * 128)
        psO = ps_small()
        for c in range(NCH):
            nc.tensor.matmul(psO[:], pn[c][:, jsl], wt["wpot"][c][:],
                             start=(c == 0), stop=(c == NCH - 1))
        dma(out[j * 128:(j + 1) * 128, :], psO[:])

    ctx.close()


def _build():
    if "nc" in _CACHE:
        return _CACHE["nc"]
    nc = bacc.Bacc("TRN2", target_bir_lowering=False, debug=False,
                   num_devices=N_CORES)
    names = {
        "hT": (D, TPC), "w1t": (D, D), "w2t": (D, D), "wgt": (D, D),
        "wpvt": (D, D), "wpot": (D, D),
        "ones128": (128, 1), "ones_r": (1, 128), "ident": (128, 128),
        "iota512": (128, TPC), "simb": (1, 1),
        "vm8": (128, 8), "st8": (128, 8), "strc": (1, 1),
        "ltri": (128, 128), "lenmask": (128, 4), "e8": (8, D),
        "wkeff": (D, H), "bias1": (128, NCH), "bias2": (128, NCH),
        "lng": (128, NCH), "lnb": (128, NCH),
    }
    _fr = {"hT", "wgt"}
    _f16 = {"w1t", "w2t", "wpvt", "wpot", "wkeff", "iota512", "e8"}
    prm = {}
    for k, sh in names.items():
        dt = FR if k in _fr else (F16 if k in _f16 else FP)
        prm[k] = nc.dram_tensor(k, list(sh), dt, kind="ExternalInput").ap()
    out = nc.dram_tensor("out", [TPC, D], FP, kind="ExternalOutput").ap()
    with tile.TileContext(nc) as tc:
        _emit(nc, tc, prm, out)
    nc.compile()
    _CACHE["nc"] = nc
    return nc


def _host_prep(inputs):
    f32 = np.float32
    f16 = np.float16
    f64 = np.float64
    hidden = np.asarray(inputs["hidden"], f32)
    lengths = np.asarray(inputs["lengths"], f32)
    consts = {
        "ones128": np.ones((128, 1), f32),
        "ones_r": np.ones((1, 128), f32),
        "ident": np.eye(128, dtype=f32),
        "ltri": np.triu(np.ones((128, 128), f32), 1),
        "simb": np.asarray(inputs["sim_bias"], f32).reshape(1, 1),
        "wkeff": np.ascontiguousarray(
            (np.asarray(inputs["Wpk"], f64).T.reshape(D, H, HD)
             * np.asarray(inputs["learned_query"],
                          f64).reshape(H, HD)[None]
             ).sum(-1).astype(f16)),
        "bias1": np.ascontiguousarray(
            np.asarray(inputs["b1"], f32).reshape(NCH, 128).T),
        "bias2": np.ascontiguousarray(
            np.asarray(inputs["b2"], f32).reshape(NCH, 128).T),
        "lng": np.ascontiguousarray(
            np.asarray(inputs["ln_g"], f32).reshape(NCH, 128).T),
        "lnb": np.ascontiguousarray(
            np.asarray(inputs["ln_b"], f32).reshape(NCH, 128).T),
    }
    for k, w in (("w1t", "W1"), ("w2t", "W2"), ("wpvt", "Wpv"),
                 ("wpot", "Wpo")):
        consts[k] = np.ascontiguousarray(np.asarray(inputs[w], f16).T)
    consts["wgt"] = np.ascontiguousarray(
        (np.asarray(inputs["Wq"], f64).T
         @ np.asarray(inputs["Wk"], f64)).astype(f32))
    consts["e8"] = np.ascontiguousarray(
        (np.arange(D)[None, :] // HD == np.arange(H)[:, None]).astype(f16))

    actual = (lengths * f32(L + 1)).astype(np.int32)
    valid = np.clip(actual - 1, 0, L)
    cut = (lengths * f32(L)).astype(np.int32)
    grid = np.arange(L).reshape(8, 128).T        # [p, c] -> token c*128+p

    in_maps = []
    for c in range(N_CORES):
        b, h = c // 2, c % 2
        tok0 = h * TPC
        hT = np.ascontiguousarray(hidden[b, tok0:tok0 + TPC, :].T)
        vm8 = ((grid < valid[b]) & (grid < L - 1)).astype(f32)
        st8 = np.zeros((128, 8), f32)
        if valid[b] < L:
            st8[valid[b] % 128, valid[b] // 128] = 1.0
        stv = 1.0 if valid[b] == 511 else 0.0
        strc = np.full((1, 1), (1.0 if 511 < valid[b] else 0.0) * (1 - stv),
                       f32)
        lm = np.zeros((128, 4), f32)
        for tch in range(4):
            g = tok0 + tch * 128 + np.arange(128)
            lm[:, tch] = (g < cut[b]).astype(f32)
        m = dict(consts)
        m.update({
            "hT": hT, "lenmask": lm, "vm8": vm8, "st8": st8, "strc": strc,
            "iota512": np.tile(np.arange(TPC, dtype=f16) + f16(512.0 * h),
                               (128, 1)),
        })
        in_maps.append(m)
    return in_maps


def kernel(**inputs):
    nc = _build()
    in_maps = _host_prep(inputs)
    res = run_bass_kernel_spmd(nc, in_maps, list(range(N_CORES)))
    out = np.empty((B, L, D), np.float32)
    for c in range(N_CORES):
        b, h = c // 2, c % 2
        out[b, h * TPC:(h + 1) * TPC, :] = res.results[c]["out"]
    return out
